# revision 1
# baseline (speedup 1.0000x reference)
"""Trainium2 Bass kernel for nn_AttnBlock_61684320305872.

Computes: GroupNorm(32 groups) -> q/k/v 1x1 convs -> full self-attention over
64x64=4096 spatial positions -> output 1x1 conv -> residual add.

Sharding (8 cores): data-parallel over (batch, spatial-half). Core c handles
batch b=c//2 and query-half h=c%2. Each core computes GroupNorm + full K/V for
its batch (K/V work duplicated across the pair of cores sharing a batch) and
Q + attention rows + projection + residual for its own 2048 positions.
The host permutes each core's spatial axis so its own positions come first;
attention is invariant to key/value ordering, so no unpermute is needed on
the K/V side.

On-chip layout avoids all transposes:
  scores are built transposed  sT[j,i] = sum_d k[d,j] q[d,i]  (lhsT = k slice)
  vT[hw,c] is produced directly by the V projection (lhsT = h_ slice)
  attention out oT[c,i] = sum_j vT[j,c]^T exp_sT[j,i]  accumulates over j
  softmax denominators via a ones-column matmul; 1/den is applied after the
  output projection (it commutes: proj contracts c, den scales per-i).

Matmuls run in float32r (~1.5e-4 rel err, 4x faster than float32 on PE).
"""
import sys

sys.path.insert(0, "/opt/trn_rl_repo")

from contextlib import ExitStack

import numpy as np

import concourse.bass as bass
import concourse.tile as tile
from concourse import bacc, mybir

F32 = mybir.dt.float32
F32R = mybir.dt.float32r
AF = mybir.ActivationFunctionType
OP = mybir.AluOpType

B, C, H, W = 4, 512, 64, 64
HW = H * W            # 4096 spatial positions
OWN = HW // 2         # 2048 query positions per core
P = 128               # partitions
CO = C // P           # 4 channel chunks
BLK = 512             # spatial block width for streamed phases
NBLK = HW // BLK      # 8
NJT = HW // P         # 32 key tiles
NIC = OWN // BLK      # 4 query chunks
G = 32                # groups
GSZ = C // G          # 16 channels per group
EPS = 1e-6
SCALE = 1.0 / float(np.sqrt(C))

_CACHED_NC = None
_LAST = None


def _build():
    nc = bacc.Bacc("TRN2", target_bir_lowering=False, debug=False, num_devices=8)

    xin = nc.dram_tensor("xin", [C, HW], F32, kind="ExternalInput")
    w_d = {n: nc.dram_tensor(n, [C, C], F32, kind="ExternalInput")
           for n in ("wq", "wk", "wv", "wo")}
    # host-prepacked constants (SBUF layouts; avoids tiny-descriptor DMAs)
    vecs_d = nc.dram_tensor("vecs", [P, 20], F32, kind="ExternalInput")
    bvbc_d = nc.dram_tensor("bvbc", [P, C], F32, kind="ExternalInput")
    emat_d = nc.dram_tensor("emat2", [P, CO * G], F32, kind="ExternalInput")
    etmat_d = nc.dram_tensor("etmat", [G, C], F32, kind="ExternalInput")
    outd = nc.dram_tensor("out", [C, OWN], F32, kind="ExternalOutput")

    x_r = xin.ap().rearrange("(co p) s -> p co s", p=P)
    out_r = outd.ap().rearrange("(co p) s -> p co s", p=P)

    with tile.TileContext(nc) as tc:
        with tc.tile_pool(name="big", bufs=1) as big, \
             tc.tile_pool(name="drp", bufs=1, space="DRAM") as drp:
            # ---- long-lived state ----
            k_sb = big.tile([P, CO, HW], F32R, name="k_sb", tag="k_sb")
            vT_sb = big.tile([P, NJT, C], F32R, name="vT_sb", tag="vT_sb")
            a_sb = big.tile([P, CO], F32, name="a_sb", tag="a_sb")
            bsh_sb = big.tile([P, CO], F32, name="bsh_sb", tag="bsh_sb")
            ones_r = big.tile([P, 1], F32R, name="ones_r", tag="ones_r")
            onesrow_r = big.tile([1, P], F32R, name="onesrow_r", tag="onesrow_r")
            q_dram = drp.tile([P, CO, OWN], F32R, name="q_dram", tag="q_dram")

            vecs_sb = big.tile([P, 20], F32, name="vecs_sb", tag="vecs_sb")
            nc.sync.dma_start(out=vecs_sb, in_=vecs_d.ap())
            bq_sb, bk_sb = vecs_sb[:, 0:4], vecs_sb[:, 4:8]
            gs_sb, gb_sb = vecs_sb[:, 12:16], vecs_sb[:, 16:20]

            with ExitStack() as ph:
                # ---- phase A+B resources (released before attention) ----
                strm = ph.enter_context(tc.tile_pool(name="strm", bufs=3))
                ps1 = ph.enter_context(tc.tile_pool(name="ps1", bufs=1, space="PSUM"))

                def issue_xb(s):
                    xb = strm.tile([P, CO, BLK], F32, name=f"xb{s}", tag="xblk",
                                   bufs=2)
                    eng = nc.sync if s % 2 == 0 else nc.scalar
                    eng.dma_start(out=xb, in_=x_r[:, :, s * BLK:(s + 1) * BLK])
                    return xb

                bv_bc = strm.tile([P, C], F32, name="bv_bc", tag="bv_bc", bufs=1)

                def load_weight(n, dst, eng):
                    # staged through the hblk ring (same slot size, no extra SBUF)
                    stage = strm.tile([P, CO, C], F32, name=f"stg_{n}", tag="hblk",
                                      bufs=2)
                    eng.dma_start(out=stage,
                                  in_=w_d[n].ap().rearrange("(eo p) d -> p eo d", p=P))
                    nc.vector.tensor_copy(out=dst, in_=stage)

                # ---- phase A: GroupNorm statistics over the full batch image.
                with tc.tile_pool(name="pa", bufs=3) as pa:
                    stats_sb = pa.tile([P, CO, NBLK, 6], F32, name="stats",
                                       tag="stats", bufs=1)
                    for s in range(NBLK):
                        xb = pa.tile([P, CO, BLK], F32, name=f"xa{s}", tag="xablk")
                        eng = nc.sync if s % 2 == 0 else nc.scalar
                        eng.dma_start(out=xb, in_=x_r[:, :, s * BLK:(s + 1) * BLK])
                        for co in range(CO):
                            nc.vector.bn_stats(out=stats_sb[:, co, s, :],
                                               in_=xb[:, co, :])
                    E_sb = pa.tile([P, CO, G], F32, name="E_sb", tag="E_sb", bufs=1)
                    Et_sb = pa.tile([P, CO, P], F32, name="Et_sb", tag="Et_sb", bufs=1)
                    eps_sb = pa.tile([P, 1], F32, name="eps_sb", tag="eps_sb", bufs=1)
                    nc.vector.memset(eps_sb, EPS)
                    nc.sync.dma_start(
                        out=E_sb, in_=emat_d.ap().rearrange("p (t g) -> p t g", g=G))
                    nc.sync.dma_start(
                        out=Et_sb[:G, :, :],
                        in_=etmat_d.ap().rearrange("g (t c) -> g t c", c=P))
                    mv = pa.tile([P, CO, 2], F32, name="mv", tag="mv", bufs=1)
                    t2 = pa.tile([P, CO, 2], F32, name="t2", tag="t2", bufs=1)
                    gw = pa.tile([G, 4], F32, name="gw", tag="gw", bufs=1)
                    gsr = pa.tile([G, 2], F32, name="gsr", tag="gsr", bufs=1)
                    mrs = pa.tile([P, CO, 2], F32, name="mrs", tag="mrs", bufs=1)
                    for co in range(CO):
                        nc.vector.bn_aggr(out=mv[:, co, :], in_=stats_sb[:, co, :, :])
                    # t2 = [mean_c, var_c + mean_c^2] per channel
                    nc.vector.tensor_copy(out=t2[:, :, 0], in_=mv[:, :, 0])
                    nc.vector.tensor_mul(out=t2[:, :, 1], in0=mv[:, :, 0], in1=mv[:, :, 0])
                    nc.vector.tensor_add(out=t2[:, :, 1], in0=t2[:, :, 1], in1=mv[:, :, 1])
                    # group sums via indicator matmul -> [32, 2]
                    psg = ps1.tile([G, 2], F32, name="psg", tag="psg", bufs=1,
                                   space="PSUM")
                    for co in range(CO):
                        nc.tensor.matmul(psg, E_sb[:, co, :], t2[:, co, :],
                                         start=(co == 0), stop=(co == CO - 1))
                    # gw: [group mean, E[var+mean^2], var_g, rstd]
                    nc.scalar.activation(out=gw[:, 0:2], in_=psg, func=AF.Copy,
                                         scale=1.0 / GSZ)
                    nc.vector.tensor_mul(out=gw[:, 2:3], in0=gw[:, 0:1], in1=gw[:, 0:1])
                    nc.vector.tensor_tensor(gw[:, 2:3], gw[:, 1:2], gw[:, 2:3],
                                            OP.subtract)
                    nc.scalar.activation(out=gw[:, 3:4], in_=gw[:, 2:3], func=AF.Sqrt,
                                         bias=eps_sb[:G], scale=1.0)
                    nc.vector.reciprocal(out=gw[:, 3:4], in_=gw[:, 3:4])
                    nc.vector.tensor_copy(out=gsr[:, 0:1], in_=gw[:, 0:1])
                    nc.vector.tensor_copy(out=gsr[:, 1:2], in_=gw[:, 3:4])
                    # broadcast group (mean, rstd) back to channels
                    for co in range(CO):
                        psb = ps1.tile([P, 2], F32, name=f"psb{co}", tag="psbc", bufs=1,
                                       space="PSUM")
                        nc.tensor.matmul(psb, Et_sb[:G, co, :], gsr, start=True,
                                         stop=True)
                        nc.vector.tensor_copy(out=mrs[:, co, :], in_=psb)
                    # h = a*x + b with a = gn_scale*rstd, b = gn_bias - a*mean
                    nc.vector.tensor_mul(out=a_sb, in0=gs_sb, in1=mrs[:, :, 1])
                    nc.vector.tensor_mul(out=bsh_sb, in0=a_sb, in1=mrs[:, :, 0])
                    nc.vector.tensor_tensor(bsh_sb, gb_sb, bsh_sb, OP.subtract)

                # ---- phase B: all projections, own-half blocks first so the
                # q spill is written long before the attention phase reads it.
                ones_f = strm.tile([P, 1], F32, name="ones_f", tag="ones_f", bufs=1)
                nc.vector.memset(ones_f, 1.0)
                nc.vector.tensor_copy(out=ones_r, in_=ones_f)
                onesrow_f = strm.tile([1, P], F32, name="onesrow_f", tag="onesrow_f",
                                      bufs=1)
                nc.vector.memset(onesrow_f, 1.0)
                nc.vector.tensor_copy(out=onesrow_r, in_=onesrow_f)

                nc.sync.dma_start(out=bv_bc, in_=bvbc_d.ap())
                pw = ph.enter_context(tc.tile_pool(name="pw", bufs=1))
                wq_sb = pw.tile([P, CO, C], F32R, name="wq_sb", tag="wq")
                wk_sb = pw.tile([P, CO, C], F32R, name="wk_sb", tag="wk")
                wv_sb = pw.tile([P, CO, C], F32R, name="wv_sb", tag="wv")
                load_weight("wv", wv_sb, nc.sync)
                load_weight("wq", wq_sb, nc.scalar)
                load_weight("wk", wk_sb, nc.sync)
                for s in range(NBLK):
                    xb = issue_xb(s)
                    hb = strm.tile([P, CO, BLK], F32R, name=f"hb{s}", tag="hblk",
                                   bufs=2)
                    for co in range(CO):
                        nc.vector.tensor_scalar(hb[:, co, :], xb[:, co, :],
                                                a_sb[:, co:co + 1],
                                                bsh_sb[:, co:co + 1],
                                                OP.mult, OP.add)
                    if s < NIC:  # own query half
                        for do in range(CO):
                            psq = ps1.tile([P, BLK], F32, name=f"psq{s}_{do}",
                                           tag="psq", bufs=2, space="PSUM")
                            for eo in range(CO):
                                nc.tensor.matmul(
                                    psq, wq_sb[:, eo, do * P:(do + 1) * P],
                                    hb[:, eo, :], start=(eo == 0),
                                    stop=(eo == CO - 1))
                            qwt = strm.tile([P, BLK], F32R, name=f"qwt{s}_{do}",
                                            tag="qwt", bufs=2)
                            nc.scalar.activation(out=qwt, in_=psq, func=AF.Identity,
                                                 bias=bq_sb[:, do:do + 1], scale=1.0)
                            nc.sync.dma_start(
                                out=q_dram[:, do, s * BLK:(s + 1) * BLK], in_=qwt)
                    for do in range(CO):
                        psk = ps1.tile([P, BLK], F32, name=f"psk{s}_{do}", tag="psk",
                                       bufs=2, space="PSUM")
                        for eo in range(CO):
                            nc.tensor.matmul(psk, wk_sb[:, eo, do * P:(do + 1) * P],
                                             hb[:, eo, :], start=(eo == 0),
                                             stop=(eo == CO - 1))
                        nc.scalar.activation(out=k_sb[:, do, s * BLK:(s + 1) * BLK],
                                             in_=psk, func=AF.Identity,
                                             bias=bk_sb[:, do:do + 1], scale=1.0)
                    for it in range(BLK // P):
                        psv = ps1.tile([P, C], F32, name=f"psv{s}_{it}", tag="psv",
                                       bufs=2, space="PSUM")
                        for eo in range(CO):
                            nc.tensor.matmul(psv, hb[:, eo, it * P:(it + 1) * P],
                                             wv_sb[:, eo, :], start=(eo == 0),
                                             stop=(eo == CO - 1))
                        nc.vector.tensor_add(out=vT_sb[:, s * (BLK // P) + it, :],
                                             in0=psv, in1=bv_bc)

            # first attention q-chunk: prefetch overlaps the tail of phase B
            qsl0 = big.tile([P, CO, BLK], F32R, name="q0", tag="qsl0")
            for co in range(CO):
                eng = nc.sync if co % 2 == 0 else nc.scalar
                eng.dma_start(out=qsl0[:, co, :], in_=q_dram[:, co, 0:BLK])

            # ---- phase C: attention + projection, per 512-wide query chunk.
            # Software-pipelined emission: chunk ic's PSUM->SBUF copies are
            # emitted before chunk ic+1's score loop (freeing the pso bank
            # ring early), and its projection/epilogue after it (so the PE
            # never waits on the DVE/ACT epilogue at a chunk boundary).
            with tc.tile_pool(name="att", bufs=1) as att, \
                 tc.tile_pool(name="ps2", bufs=1, space="PSUM") as ps2:

                # wo arrives here: staged through the osc ring, rounded to f32r
                # (needed only by the first projection ~85us into phase C)
                wo_sb = att.tile([P, CO, C], F32R, name="wo_sb", tag="wo_sb")
                wo_stage = att.tile([P, CO, C], F32, name="wo_stage", tag="osc",
                                    bufs=2)
                nc.scalar.dma_start(out=wo_stage,
                                    in_=w_d["wo"].ap().rearrange("(eo p) d -> p eo d",
                                                                 p=P))
                nc.vector.tensor_copy(out=wo_sb, in_=wo_stage)

                def emit_jloop(ic, qsl):
                    pso = [ps2.tile([P, BLK], F32, name=f"pso{ic}_{ct}", tag="pso",
                                    bufs=CO, space="PSUM") for ct in range(CO)]
                    psd = ps2.tile([1, BLK], F32, name=f"psd{ic}", tag="psd", bufs=2,
                                   space="PSUM")
                    et_prev = None
                    for j in range(NJT):
                        pss = ps2.tile([P, BLK], F32, name=f"pss{ic}_{j}", tag="pss",
                                       bufs=2, space="PSUM")
                        for co in range(CO):
                            nc.tensor.matmul(pss, k_sb[:, co, j * P:(j + 1) * P],
                                             qsl[:, co, :], start=(co == 0),
                                             stop=(co == CO - 1))
                        et = att.tile([P, BLK], F32R, name=f"e{ic}_{j}", tag="exp",
                                      bufs=3)
                        nc.scalar.activation(out=et, in_=pss, func=AF.Exp, scale=SCALE)
                        for ct in range(CO):
                            nc.tensor.matmul(pso[ct], vT_sb[:, j, ct * P:(ct + 1) * P],
                                             et, start=(j == 0), stop=(j == NJT - 1))
                        if j % 2 == 0:
                            et_prev = et
                        else:
                            # tree-sum on DVE: quarters the denominator matmuls
                            es = att.tile([P, BLK], F32R, name=f"es{ic}_{j}",
                                          tag="esum", bufs=3)
                            nc.vector.tensor_add(out=es, in0=et_prev, in1=et)
                            if j % 4 == 1:
                                es_prev = es
                            else:
                                es2 = att.tile([P, BLK], F32R, name=f"es2_{ic}_{j}",
                                               tag="esum", bufs=3)
                                nc.vector.tensor_add(out=es2, in0=es_prev, in1=es)
                                nc.tensor.matmul(psd, ones_r, es2, start=(j == 3),
                                                 stop=(j == NJT - 1))
                    return pso, psd

                def emit_copies(ic, pso):
                    # free the pso bank ring: 2 copies on DVE, 2 on ScalarE
                    osc = att.tile([P, CO, BLK], F32R, name=f"osc{ic}", tag="osc",
                                   bufs=2)
                    nc.vector.tensor_copy(out=osc[:, 0, :], in_=pso[0])
                    nc.scalar.activation(out=osc[:, 1, :], in_=pso[1], func=AF.Identity)
                    nc.vector.tensor_copy(out=osc[:, 2, :], in_=pso[2])
                    nc.scalar.activation(out=osc[:, 3, :], in_=pso[3], func=AF.Identity)
                    return osc

                def emit_tail(ic, psd, osc):
                    den = att.tile([1, BLK], F32R, name=f"den{ic}", tag="den", bufs=2)
                    with nc.allow_low_precision(reason="1/den rounded to f32r for "
                                                "the broadcast matmul; ~1e-4 is "
                                                "within kernel tolerance"):
                        nc.vector.reciprocal(out=den, in_=psd)
                    # broadcast 1/den to all partitions with a K=1 matmul
                    # (SBUF APs cannot have a zero partition step, and a DRAM
                    # bounce costs ~6us of latency on the final chunk)
                    rbc_ps = ps2.tile([P, BLK], F32, name=f"rbcp{ic}", tag="psd",
                                      bufs=2, space="PSUM")
                    nc.tensor.matmul(rbc_ps, onesrow_r, den, start=True, stop=True)
                    rbc = att.tile([P, BLK], F32, name=f"rbc{ic}", tag="rbc", bufs=2)
                    nc.vector.tensor_copy(out=rbc, in_=rbc_ps)
                    for dt_ in range(CO):
                        psy = ps2.tile([P, BLK], F32, name=f"psy{ic}_{dt_}", tag="pss",
                                       bufs=2, space="PSUM")
                        for ct in range(CO):
                            nc.tensor.matmul(psy, wo_sb[:, ct, dt_ * P:(dt_ + 1) * P],
                                             osc[:, ct, :], start=(ct == 0),
                                             stop=(ct == CO - 1))
                        xr = att.tile([P, BLK], F32, name=f"xr{ic}_{dt_}", tag="xres",
                                      bufs=2)
                        nc.sync.dma_start(out=xr,
                                          in_=x_r[:, dt_, ic * BLK:(ic + 1) * BLK])
                        y = att.tile([P, BLK], F32, name=f"y{ic}_{dt_}", tag="y", bufs=2)
                        nc.vector.tensor_mul(out=y, in0=psy, in1=rbc)
                        nc.vector.tensor_scalar_add(y, y, vecs_sb[:, 8 + dt_:9 + dt_])
                        nc.vector.tensor_add(out=y, in0=y, in1=xr)
                        nc.sync.dma_start(out=out_r[:, dt_, ic * BLK:(ic + 1) * BLK],
                                          in_=y)

                def load_qsl(ic):
                    t = att.tile([P, CO, BLK], F32R, name=f"q{ic}", tag="qsl", bufs=2)
                    for co in range(CO):
                        eng = nc.sync if co % 2 == 0 else nc.scalar
                        eng.dma_start(out=t[:, co, :],
                                      in_=q_dram[:, co, ic * BLK:(ic + 1) * BLK])
                    return t

                qsl = qsl0
                prev = None
                for ic in range(NIC):
                    if prev is not None:
                        osc_p = emit_copies(prev[0], prev[1])
                    cur = (ic, *emit_jloop(ic, qsl))
                    if ic + 1 < NIC:
                        qsl = load_qsl(ic + 1)
                    if prev is not None:
                        emit_tail(prev[0], prev[2], osc_p)
                    prev = cur
                osc_p = emit_copies(prev[0], prev[1])
                emit_tail(prev[0], prev[2], osc_p)

    nc.compile()
    return nc


def _make_in_maps(inputs):
    x = np.asarray(inputs["x"], np.float32).reshape(B, C, HW)
    rep = {
        "wq": np.ascontiguousarray(np.asarray(inputs["wq"], np.float32)),
        "wk": np.ascontiguousarray(np.asarray(inputs["wk"], np.float32)),
        "wv": np.ascontiguousarray(np.asarray(inputs["wv"], np.float32)),
        "wo": np.ascontiguousarray(np.asarray(inputs["wo"], np.float32)),
        "bq": np.asarray(inputs["bq"], np.float32),
        "bk": np.asarray(inputs["bk"], np.float32),
        "bv": np.asarray(inputs["bv"], np.float32),
        "bo": np.asarray(inputs["bo"], np.float32),
        "gsc": np.asarray(inputs["gn_scale"], np.float32),
        "gbi": np.asarray(inputs["gn_bias"], np.float32),
    }
    emat = np.zeros((C, G), np.float32)
    emat[np.arange(C), np.arange(C) // GSZ] = 1.0
    # emat2[p, t*G+g] = emat[t*P+p, g]; etmat[g, t*P+c] = emat[t*P+c, g]
    rep["emat2"] = np.ascontiguousarray(
        emat.reshape(CO, P, G).transpose(1, 0, 2).reshape(P, CO * G))
    rep["etmat"] = np.ascontiguousarray(emat.T)
    vecs = np.zeros((P, 20), np.float32)
    for i, nm in enumerate(("bq", "bk", "bo", "gsc", "gbi")):
        vecs[:, 4 * i:4 * i + 4] = rep[nm].reshape(CO, P).T
    rep["vecs"] = vecs
    rep["bvbc"] = np.ascontiguousarray(np.broadcast_to(rep["bv"], (P, C)))
    for nm in ("bq", "bk", "bo", "gsc", "gbi", "bv"):
        del rep[nm]
    in_maps = []
    for core in range(8):
        b, half = core // 2, core % 2
        xb = x[b]
        own = xb[:, half * OWN:(half + 1) * OWN]
        oth = xb[:, (1 - half) * OWN:(2 - half) * OWN]
        xp = np.ascontiguousarray(np.concatenate([own, oth], axis=1))
        in_maps.append({"xin": xp, **rep})
    return in_maps


def kernel(**inputs):
    global _CACHED_NC, _LAST
    from concourse.bass_utils import run_bass_kernel_spmd

    if _CACHED_NC is None:
        _CACHED_NC = _build()
    in_maps = _make_in_maps(inputs)
    res = run_bass_kernel_spmd(_CACHED_NC, in_maps, core_ids=list(range(8)))
    _LAST = res
    out = np.empty((B, C, HW), np.float32)
    for core in range(8):
        b, half = core // 2, core % 2
        out[b][:, half * OWN:(half + 1) * OWN] = res.results[core]["out"]
    return out.reshape(B, C, H, W)



# revision 8
# speedup vs baseline: 1.5611x; 1.5611x over previous
"""Trainium2 Bass kernel for nn_AttnBlock_61684320305872.

Computes: GroupNorm(32 groups) -> q/k/v 1x1 convs -> full self-attention over
64x64=4096 spatial positions -> output 1x1 conv -> residual add.

Sharding (8 cores): data-parallel over (batch, spatial-half). Core c handles
batch b=c//2 and query-half h=c%2. Each core computes GroupNorm + full K/V/U
for its batch and attention rows + residual for its own 2048 positions. The
host permutes each core's spatial axis so its own positions come first.

All heavy matmuls run in fp8 e4m3 with DoubleRow perf mode (256-deep
contraction per instruction, 2x f32r PE throughput):
  - GroupNorm is folded into the projection weights: w' = q8(16*a (.) w) with
    a = gn_scale*rstd per input channel, so the PE consumes q8(x) directly and
    no normalized activation tensor is ever materialized. The GroupNorm shift
    b = gn_bias - a*mean contributes a per-output-channel bias: negligible for
    q/k (score shift ~0.4% of fp8 noise, dropped), exact for v (folded into
    the epilogue constant via const = wo^T(bias'_v + bv) + bo).
  - The output projection is fused into attention: U = q8(wo^T V) is
    precomputed (same layout as the transposed-V of a plain kernel), so the
    attention matmul y[d,i] = sum_j U[d,j] e[j,i] directly produces the
    projected numerator; the epilogue is y/den + const + x.
  - exp runs on ACT reading a 2-bank PSUM score pair [128,2,512] and writing
    an fp8 pair tile that feeds DoubleRow directly; exp carries a -2.5 shift
    so e^(s-2.5) stays under the fp8e4 max of 240 (max observed score ~7.06),
    which cancels in softmax normalization.
  - softmax denominators come from a ones-lhsT DoubleRow matmul per pair.

End-to-end numpy emulation of this exact pipeline: max-rel 1.03e-2 (gate 2e-2).
"""
import sys

sys.path.insert(0, "/opt/trn_rl_repo")

from contextlib import ExitStack

import numpy as np
import ml_dtypes

import concourse.bass as bass
import concourse.tile as tile
from concourse import bacc, mybir

F32 = mybir.dt.float32
F32R = mybir.dt.float32r
FP8 = mybir.dt.float8e4
AF = mybir.ActivationFunctionType
OP = mybir.AluOpType
DR = mybir.MatmulPerfMode.DoubleRow

B, C, H, W = 4, 512, 64, 64
HW = H * W            # 4096 spatial positions
OWN = HW // 2         # 2048 query positions per core
P = 128               # partitions
CO = C // P           # 4 channel chunks
BLK = 512             # block width
NBLK = HW // BLK      # 8
NJT = HW // P         # 32 key tiles
NPAIR = NJT // 2      # 16 key-tile pairs per chunk
NIC = OWN // BLK      # 4 query chunks
G = 32                # groups
GSZ = C // G          # 16 channels per group
EPS = 1e-6
SCALE = 1.0 / float(np.sqrt(C))
SHIFT = -2.5          # exp shift: e^(s+SHIFT) <= ~117 < 240 (fp8e4 max)
WS = 16.0             # weight pre-scale before fp8 quantization

_CACHED_NC = None
_LAST = None


def _build():
    nc = bacc.Bacc("TRN2", target_bir_lowering=False, debug=False, num_devices=8)

    xin = nc.dram_tensor("xin", [C, HW], F32, kind="ExternalInput")
    w_d = {n: nc.dram_tensor(n, [C, C], F32, kind="ExternalInput")
           for n in ("wq", "wk", "wv")}
    wo8_d = nc.dram_tensor("wo8", [P, CO, C], FP8, kind="ExternalInput")
    vecs_d = nc.dram_tensor("vecs", [P, 24], F32, kind="ExternalInput")
    emat_d = nc.dram_tensor("emat2", [P, CO * G], F32, kind="ExternalInput")
    etmat_d = nc.dram_tensor("etmat", [G, C], F32, kind="ExternalInput")
    outd = nc.dram_tensor("out", [C, OWN], F32, kind="ExternalOutput")

    x_r = xin.ap().rearrange("(co p) s -> p co s", p=P)
    out_r = outd.ap().rearrange("(co p) s -> p co s", p=P)

    with tile.TileContext(nc) as tc:
        with tc.tile_pool(name="big", bufs=1) as big, \
             tc.tile_pool(name="drp", bufs=1, space="DRAM") as drp:
            # ---- long-lived state ----
            x8_sb = big.tile([P, CO, HW], FP8, name="x8_sb", tag="x8")
            k8_sb = big.tile([P, CO, HW], FP8, name="k8_sb", tag="k8")
            V8_sb = big.tile([P, CO, HW], FP8, name="V8_sb", tag="V8")
            uT8_sb = big.tile([P, NJT, C], FP8, name="uT8_sb", tag="uT8")
            q8_sb = big.tile([P, CO, OWN], FP8, name="q8_sb", tag="q8")
            wk8 = big.tile([P, CO, C], FP8, name="wk8", tag="wk8")
            wv8 = big.tile([P, CO, C], FP8, name="wv8", tag="wv8")
            wq8 = big.tile([P, CO, C], FP8, name="wq8", tag="wq8")
            wo8_sb = big.tile([P, CO, C], FP8, name="wo8_sb", tag="wo8")
            vecs_sb = big.tile([P, 24], F32, name="vecs_sb", tag="vecs")
            a_sb = big.tile([P, CO], F32, name="a_sb", tag="a_sb")
            bsh_sb = big.tile([P, CO], F32, name="bsh_sb", tag="bsh")
            a16_sb = big.tile([P, CO], F32, name="a16_sb", tag="a16")
            constx = big.tile([P, CO], F32, name="constx", tag="constx")
            ones2p = big.tile([P, 2, 16], FP8, name="ones2p", tag="ones2p")
            onesrow_r = big.tile([1, P], F32R, name="onesrow_r", tag="onesrow")
            shift_sb = big.tile([P, 1], F32, name="shift_sb", tag="shift")
            bv8p = big.tile([P, CO, 16], FP8, name="bv8p", tag="bv8p")

            nc.scalar.dma_start(out=vecs_sb, in_=vecs_d.ap())
            nc.scalar.dma_start(out=wo8_sb, in_=wo8_d.ap())
            bq_v, bk_v = vecs_sb[:, 0:4], vecs_sb[:, 4:8]
            bo_v, gs_v = vecs_sb[:, 8:12], vecs_sb[:, 12:16]
            gb_v, bv_v = vecs_sb[:, 16:20], vecs_sb[:, 20:24]

            nc.vector.memset(shift_sb, SHIFT)
            seed = big.tile([P, 2, 16], F32, name="seed", tag="seed")
            nc.vector.memset(seed, 1.0)
            nc.vector.tensor_copy(out=ones2p, in_=seed)
            onesrow_f = big.tile([1, P], F32, name="onesrow_f", tag="onesrowf")
            nc.vector.memset(onesrow_f, 1.0)
            nc.vector.tensor_copy(out=onesrow_r, in_=onesrow_f)

            with ExitStack() as ph:
                pa = ph.enter_context(tc.tile_pool(name="pa", bufs=1))
                ps1 = ph.enter_context(tc.tile_pool(name="ps1", bufs=1,
                                                    space="PSUM"))

                # ---- phase A: stream x, GroupNorm stats + fp8 cast ----
                stats_sb = pa.tile([P, CO, NBLK, 6], F32, name="stats",
                                   tag="stats")
                for s in range(NBLK):
                    xb = pa.tile([P, CO, BLK], F32, name=f"xa{s}", tag="xablk",
                                 bufs=3)
                    nc.sync.dma_start(out=xb, in_=x_r[:, :, s * BLK:(s + 1) * BLK])
                    for co in range(CO):
                        nc.vector.bn_stats(out=stats_sb[:, co, s, :],
                                           in_=xb[:, co, :])
                        nc.scalar.activation(
                            out=x8_sb[:, co, s * BLK:(s + 1) * BLK],
                            in_=xb[:, co, :], func=AF.Copy)

                # weights stream in behind x on the same queue
                wkst = pa.tile([P, CO, C], F32, name="wkst", tag="wkst")
                wvst = pa.tile([P, CO, C], F32, name="wvst", tag="wvst")
                wqst = pa.tile([P, CO, C], F32, name="wqst", tag="wqst")
                for n, dst in (("wk", wkst), ("wv", wvst), ("wq", wqst)):
                    nc.sync.dma_start(
                        out=dst, in_=w_d[n].ap().rearrange("(eo p) d -> p eo d",
                                                           p=P))

                E_sb = pa.tile([P, CO, G], F32, name="E_sb", tag="E_sb")
                Et_sb = pa.tile([P, CO, P], F32, name="Et_sb", tag="Et_sb")
                eps_sb = pa.tile([P, 1], F32, name="eps_sb", tag="eps_sb")
                nc.vector.memset(eps_sb, EPS)
                nc.scalar.dma_start(
                    out=E_sb, in_=emat_d.ap().rearrange("p (t g) -> p t g", g=G))
                nc.scalar.dma_start(
                    out=Et_sb[:G, :, :],
                    in_=etmat_d.ap().rearrange("g (t c) -> g t c", c=P))

                # ---- stats tail: per-channel -> per-group -> a, b ----
                mv = pa.tile([P, CO, 2], F32, name="mv", tag="mv")
                t2 = pa.tile([P, CO, 2], F32, name="t2", tag="t2")
                gw = pa.tile([G, 4], F32, name="gw", tag="gw")
                gsr = pa.tile([G, 2], F32, name="gsr", tag="gsr")
                mrs = pa.tile([P, CO, 2], F32, name="mrs", tag="mrs")
                for co in range(CO):
                    nc.vector.bn_aggr(out=mv[:, co, :], in_=stats_sb[:, co, :, :])
                nc.vector.tensor_copy(out=t2[:, :, 0], in_=mv[:, :, 0])
                nc.vector.tensor_mul(out=t2[:, :, 1], in0=mv[:, :, 0],
                                     in1=mv[:, :, 0])
                nc.vector.tensor_add(out=t2[:, :, 1], in0=t2[:, :, 1],
                                     in1=mv[:, :, 1])
                psg = ps1.tile([G, 2], F32, name="psg", tag="psg", space="PSUM")
                for co in range(CO):
                    nc.tensor.matmul(psg, E_sb[:, co, :], t2[:, co, :],
                                     start=(co == 0), stop=(co == CO - 1))
                nc.scalar.activation(out=gw[:, 0:2], in_=psg, func=AF.Copy,
                                     scale=1.0 / GSZ)
                nc.vector.tensor_mul(out=gw[:, 2:3], in0=gw[:, 0:1],
                                     in1=gw[:, 0:1])
                nc.vector.tensor_tensor(gw[:, 2:3], gw[:, 1:2], gw[:, 2:3],
                                        OP.subtract)
                nc.scalar.activation(out=gw[:, 3:4], in_=gw[:, 2:3], func=AF.Sqrt,
                                     bias=eps_sb[:G], scale=1.0)
                nc.vector.reciprocal(out=gw[:, 3:4], in_=gw[:, 3:4])
                nc.vector.tensor_copy(out=gsr[:, 0:1], in_=gw[:, 0:1])
                nc.vector.tensor_copy(out=gsr[:, 1:2], in_=gw[:, 3:4])
                for co in range(CO):
                    psb = ps1.tile([P, 2], F32, name=f"psb{co}", tag="psbc",
                                   space="PSUM")
                    nc.tensor.matmul(psb, Et_sb[:G, co, :], gsr, start=True,
                                     stop=True)
                    nc.vector.tensor_copy(out=mrs[:, co, :], in_=psb)
                # a = gn_scale*rstd, b = gn_bias - a*mean
                nc.vector.tensor_mul(out=a_sb, in0=gs_v, in1=mrs[:, :, 1])
                nc.vector.tensor_mul(out=bsh_sb, in0=a_sb, in1=mrs[:, :, 0])
                nc.vector.tensor_tensor(bsh_sb, gb_v, bsh_sb, OP.subtract)
                nc.vector.tensor_scalar_mul(a16_sb, a_sb, WS)

                # ---- fold GroupNorm scale into fp8 weights (k first) ----
                for wst, w8 in ((wkst, wk8), (wvst, wv8), (wqst, wq8)):
                    for co in range(CO):
                        if co < 2:
                            nc.scalar.activation(out=w8[:, co, :],
                                                 in_=wst[:, co, :], func=AF.Copy,
                                                 scale=a16_sb[:, co:co + 1])
                        else:
                            nc.vector.tensor_scalar_mul(w8[:, co, :],
                                                        wst[:, co, :],
                                                        a16_sb[:, co:co + 1])

                # ---- bias'_v[c] = sum_e wv[e,c]*b[e] (exact, off critical) ----
                bvt = pa.tile([P, CO], F32, name="bvt", tag="bvt")
                for co in range(CO):
                    psbv = ps1.tile([P, 1], F32, name=f"psbv{co}", tag="psbv",
                                    space="PSUM")
                    for eo in range(CO):
                        nc.tensor.matmul(
                            psbv,
                            wvst[:, eo, co * P:(co + 1) * P],
                            bsh_sb[:, eo:eo + 1],
                            start=(eo == 0), stop=(eo == CO - 1))
                    nc.vector.tensor_copy(out=bvt[:, co:co + 1], in_=psbv)
                nc.vector.tensor_add(out=bvt, in0=bvt, in1=bv_v)
                for co in range(CO):
                    nc.vector.tensor_scalar_mul(bv8p[:, co, 0:1],
                                                bvt[:, co:co + 1], 256.0)

            # ---- phase B: projections + U = wo^T V, all fp8 DoubleRow ----
            with ExitStack() as pb_ctx:
                pb = pb_ctx.enter_context(tc.tile_pool(name="pb", bufs=1))
                ps2 = pb_ctx.enter_context(tc.tile_pool(name="ps2", bufs=1,
                                                        space="PSUM"))

                def drain_act(dst, src, bias_ap):
                    if bias_ap is None:
                        nc.scalar.activation(out=dst, in_=src, func=AF.Copy,
                                             scale=1.0 / WS)
                    else:
                        nc.scalar.activation(out=dst, in_=src, func=AF.Identity,
                                             scale=1.0 / WS, bias=bias_ap)

                def drain_dve(dst, src, bias_ap):
                    if bias_ap is None:
                        nc.vector.tensor_scalar_mul(dst, src, 1.0 / WS)
                    else:
                        nc.vector.tensor_scalar(dst, src, 1.0 / WS, bias_ap,
                                                OP.mult, OP.add)

                for s in range(NBLK):
                    sl = slice(s * BLK, (s + 1) * BLK)
                    xs = x8_sb[:, :, sl]
                    for do in range(CO):
                        psk = ps2.tile([P, BLK], F32, name=f"psk{s}_{do}",
                                       tag="psk", bufs=2, space="PSUM")
                        for cp in range(2):
                            nc.tensor.matmul(
                                psk, wk8[:, 2 * cp:2 * cp + 2,
                                         do * P:(do + 1) * P],
                                xs[:, 2 * cp:2 * cp + 2, :],
                                start=(cp == 0), stop=(cp == 1), perf_mode=DR)
                        dr = drain_act if do < 2 else drain_dve
                        dr(k8_sb[:, do, sl], psk, bk_v[:, do:do + 1])
                    for do in range(CO):
                        psv = ps2.tile([P, BLK], F32, name=f"psv{s}_{do}",
                                       tag="psv", bufs=2, space="PSUM")
                        for cp in range(2):
                            nc.tensor.matmul(
                                psv, wv8[:, 2 * cp:2 * cp + 2,
                                         do * P:(do + 1) * P],
                                xs[:, 2 * cp:2 * cp + 2, :],
                                start=(cp == 0), stop=(cp == 1), perf_mode=DR)
                        dr = drain_dve if do < 2 else drain_act
                        dr(V8_sb[:, do, sl], psv, None)
                    if s < NIC:
                        for do in range(CO):
                            psq = ps2.tile([P, BLK], F32, name=f"psq{s}_{do}",
                                           tag="pmix", bufs=3, space="PSUM")
                            for cp in range(2):
                                nc.tensor.matmul(
                                    psq, wq8[:, 2 * cp:2 * cp + 2,
                                             do * P:(do + 1) * P],
                                    xs[:, 2 * cp:2 * cp + 2, :],
                                    start=(cp == 0), stop=(cp == 1),
                                    perf_mode=DR)
                            dr = drain_act if do < 2 else drain_dve
                            dr(q8_sb[:, do, sl], psq, bq_v[:, do:do + 1])
                    for jt in range(BLK // P):
                        jg = s * (BLK // P) + jt
                        psu = ps2.tile([P, C], F32, name=f"psu{s}_{jt}",
                                       tag="pmix", bufs=3, space="PSUM")
                        for cp in range(2):
                            nc.tensor.matmul(
                                psu, V8_sb[:, 2 * cp:2 * cp + 2,
                                           jg * P:(jg + 1) * P],
                                wo8_sb[:, 2 * cp:2 * cp + 2, :],
                                start=(cp == 0), stop=(cp == 1), perf_mode=DR)
                        dr = drain_dve if jt < 2 else drain_act
                        dr(uT8_sb[:, jg, :], psu, None)

                # epilogue constant: const = wo^T (bias'_v + bv) + bo, computed
                # directly in [P, CO] layout (lhsT = wo d-slice pairs)
                for co in range(CO):
                    psco = ps2.tile([P, 1], F32, name=f"psco{co}", tag="pmix",
                                    bufs=3, space="PSUM")
                    for cp in range(2):
                        nc.tensor.matmul(
                            psco, wo8_sb[:, 2 * cp:2 * cp + 2,
                                         co * P:(co + 1) * P],
                            bv8p[:, 2 * cp:2 * cp + 2, 0:1],
                            start=(cp == 0), stop=(cp == 1), perf_mode=DR)
                    nc.scalar.activation(out=constx[:, co:co + 1], in_=psco,
                                         func=AF.Identity,
                                         scale=1.0 / (WS * 256.0),
                                         bias=bo_v[:, co:co + 1])

            # ---- phase C: attention, fused projection, 2-pair-lag pipeline ----
            with tc.tile_pool(name="pc", bufs=1) as pc, \
                 tc.tile_pool(name="ps3", bufs=1, space="PSUM") as ps3:

                def emit_pair(ic, p, pso, psd, et_ring):
                    qs = q8_sb[:, :, ic * BLK:(ic + 1) * BLK]
                    pss = ps3.tile([P, 2, BLK], F32, name=f"pss{ic}_{p}",
                                   tag="pss", bufs=1, space="PSUM")
                    for t in range(2):
                        jt = 2 * p + t
                        for cp in range(2):
                            nc.tensor.matmul(
                                pss[:, t, :],
                                k8_sb[:, 2 * cp:2 * cp + 2, jt * P:(jt + 1) * P],
                                qs[:, 2 * cp:2 * cp + 2, :],
                                start=(cp == 0), stop=(cp == 1), perf_mode=DR)
                    et2 = pc.tile([P, 2, BLK], FP8, name=f"et{ic}_{p}",
                                  tag="et2", bufs=4)
                    nc.scalar.activation(out=et2, in_=pss, func=AF.Exp,
                                         scale=SCALE, bias=shift_sb)
                    nc.tensor.matmul(psd, ones2p[:, :, 0:1], et2,
                                     start=(p == 0), stop=(p == NPAIR - 1),
                                     perf_mode=DR)
                    et_ring[p] = et2

                def emit_yacc(ic, p, pso, et_ring):
                    et2 = et_ring[p]
                    for ct in range(CO):
                        nc.tensor.matmul(
                            pso[ct],
                            uT8_sb[:, 2 * p:2 * p + 2, ct * P:(ct + 1) * P],
                            et2, start=(p == 0), stop=(p == NPAIR - 1),
                            perf_mode=DR)

                def emit_epilogue(ic, pso, psd):
                    den_r = pc.tile([1, BLK], F32, name=f"den{ic}", tag="den",
                                    bufs=2)
                    nc.vector.reciprocal_approx_fast(out=den_r, in_=psd)
                    den_rr = pc.tile([1, BLK], F32R, name=f"denr{ic}",
                                     tag="denr", bufs=2)
                    nc.vector.tensor_copy(out=den_rr, in_=den_r)
                    rbc_ps = ps3.tile([P, BLK], F32, name=f"rbcp{ic}", tag="psd",
                                      bufs=2, space="PSUM")
                    nc.tensor.matmul(rbc_ps, onesrow_r, den_rr,
                                     start=True, stop=True)
                    rbc = pc.tile([P, BLK], F32, name=f"rbc{ic}", tag="rbc",
                                  bufs=2)
                    nc.vector.tensor_copy(out=rbc, in_=rbc_ps)
                    for do in range(CO):
                        xr = pc.tile([P, BLK], F32, name=f"xr{ic}_{do}",
                                     tag="xres", bufs=2)
                        eng = nc.sync if do % 2 == 0 else nc.scalar
                        eng.dma_start(out=xr,
                                      in_=x_r[:, do, ic * BLK:(ic + 1) * BLK])
                        y = pc.tile([P, BLK], F32, name=f"y{ic}_{do}", tag="y",
                                    bufs=2)
                        nc.vector.tensor_tensor(y, pso[do], rbc, OP.mult)
                        nc.vector.scalar_tensor_tensor(y, y, constx[:, do:do + 1],
                                                       xr, OP.add, OP.add)
                        eng.dma_start(out=out_r[:, do, ic * BLK:(ic + 1) * BLK],
                                      in_=y)

                prev = None
                for ic in range(NIC):
                    pso = [ps3.tile([P, BLK], F32, name=f"pso{ic}_{ct}",
                                    tag="pso", bufs=4, space="PSUM")
                           for ct in range(CO)]
                    psd = ps3.tile([1, BLK], F32, name=f"psd{ic}", tag="psd",
                                   bufs=2, space="PSUM")
                    et_ring = {}
                    for p in range(NPAIR):
                        emit_pair(ic, p, pso, psd, et_ring)
                        if p == 1 and prev is not None:
                            emit_epilogue(*prev)
                        if p >= 2:
                            emit_yacc(ic, p - 2, pso, et_ring)
                    emit_yacc(ic, NPAIR - 2, pso, et_ring)
                    emit_yacc(ic, NPAIR - 1, pso, et_ring)
                    prev = (ic, pso, psd)
                emit_epilogue(*prev)

    nc.compile()
    return nc


def _make_in_maps(inputs):
    x = np.asarray(inputs["x"], np.float32).reshape(B, C, HW)
    wo = np.asarray(inputs["wo"], np.float32)
    wo8 = np.ascontiguousarray(
        (WS * wo).reshape(CO, P, C).transpose(1, 0, 2)).astype(
            ml_dtypes.float8_e4m3)
    rep = {
        "wq": np.ascontiguousarray(np.asarray(inputs["wq"], np.float32)),
        "wk": np.ascontiguousarray(np.asarray(inputs["wk"], np.float32)),
        "wv": np.ascontiguousarray(np.asarray(inputs["wv"], np.float32)),
        "wo8": wo8,
    }
    emat = np.zeros((C, G), np.float32)
    emat[np.arange(C), np.arange(C) // GSZ] = 1.0
    rep["emat2"] = np.ascontiguousarray(
        emat.reshape(CO, P, G).transpose(1, 0, 2).reshape(P, CO * G))
    rep["etmat"] = np.ascontiguousarray(emat.T)
    vecs = np.zeros((P, 24), np.float32)
    for i, nm in enumerate(("bq", "bk", "bo", "gn_scale", "gn_bias", "bv")):
        vecs[:, 4 * i:4 * i + 4] = np.asarray(
            inputs[nm], np.float32).reshape(CO, P).T
    rep["vecs"] = vecs
    in_maps = []
    for core in range(8):
        b, half = core // 2, core % 2
        xb = x[b]
        own = xb[:, half * OWN:(half + 1) * OWN]
        oth = xb[:, (1 - half) * OWN:(2 - half) * OWN]
        xp = np.ascontiguousarray(np.concatenate([own, oth], axis=1))
        in_maps.append({"xin": xp, **rep})
    return in_maps


def kernel(**inputs):
    global _CACHED_NC, _LAST
    from concourse.bass_utils import run_bass_kernel_spmd

    if _CACHED_NC is None:
        _CACHED_NC = _build()
    in_maps = _make_in_maps(inputs)
    res = run_bass_kernel_spmd(_CACHED_NC, in_maps, core_ids=list(range(8)))
    _LAST = res
    out = np.empty((B, C, HW), np.float32)
    for core in range(8):
        b, half = core // 2, core % 2
        out[b][:, half * OWN:(half + 1) * OWN] = res.results[core]["out"]
    return out.reshape(B, C, H, W)
